# revision 7
# baseline (speedup 1.0000x reference)
"""Trainium2 Bass kernel for windowed sparse attention with dynamic position bias.

Reference computation (B_=256 windows, N=256 tokens, DIM=256, NH=8 heads, hd=32):
  qh = (q @ q_w.T + q_b)  -> heads;  kh, vh from kv projection of k
  attn = softmax(qh*s @ kh^T + rpb[h] + mask[b%64]);  out = (attn @ vh) @ proj_w.T + proj_b

Sharding: 8 cores, core c handles mask groups g in [8c, 8c+8), windows b = g + 64k
(4 windows per group -> exp(bias) tiles reused 4x per core).

Device kernel (per core, 32 windows): bf16 matmuls / fp32 PSUM.
  - projections from channel-major qT/kT (host-marshalled layout)
  - S = qh^T k (S-layout [i, j]), ACT exp from PSUM
  - P*E bias-multiply fused with row-sum via DVE tensor_tensor_reduce
  - normalize by 1/rowsum, DMA-xbar transpose P -> Pt, O^T = vh^T-packed matmuls
  - out-proj with K=1 ones-matmul bias add; output stored f16 (halves the
    device->host wire traffic vs f32; |out| <= ~2 so f16 rounding is ~5e-4 rel).

The axon host<->device link (~70 MiB/s up, ~37 MiB/s down, serialized) is the
bottleneck -- device exec is ~0.12 s. So the dispatch layer is organized around
wire bytes and overlap:
  - the Bass module + jit(shard_map(bass_exec)) wrapper are built once per
    process and cached (no per-call rebuild/recompile);
  - inputs are packed into 3 bf16 blob tensors (consts+mask+rpb | kT | qT),
    each device_put as soon as marshalled -- device_put is async under axon,
    so marshalling core i+1 overlaps the wire transfer of blob i;
  - donated output buffers are created on-device (jnp.zeros), not shipped;
  - the f16 output is fetched per-shard with copy_to_host_async, overlapping
    the f16->f32 cast + window scatter of shard c with the wire transfer of
    shard c+1.
"""

import os
import time
from contextlib import ExitStack

import numpy as np
import ml_dtypes

import jax
import jax.numpy as jnp
from jax.sharding import Mesh, PartitionSpec, NamedSharding
from jax.experimental.shard_map import shard_map  # same shim bass2jax uses

import concourse.bass as bass
import concourse.tile as tile
import concourse.mybir as mybir
from concourse import bacc
from concourse import bass2jax

BF16 = mybir.dt.bfloat16
F32 = mybir.dt.float32
F16 = mybir.dt.float16
NPBF16 = ml_dtypes.bfloat16

DIM = 256
NH = 8
HD = DIM // NH  # 32
B_ = 256
N = 256
NG = 64
NCORES = 8
GPC = NG // NCORES  # 8 groups per core
WPC = B_ // NCORES  # 32 windows per core
PD = DIM // 16  # 16

# window order: core c handles mask groups [8c, 8c+8); window b = g + 64k
_PERM = np.concatenate([
    ((np.arange(GPC) + GPC * c)[:, None] + NG * np.arange(4)[None, :]).reshape(-1)
    for c in range(NCORES)
])  # (256,) global row i of the concatenated per-core input = window _PERM[i]

# ---- packed input blobs (bf16 elements, per-core) ----
E_QT = WPC * DIM * N          # 2097152
E_KT = E_QT
E_MK = GPC * N * N            # 524288
E_RPB = NH * N * N            # 524288  stored [it, h, 128, j]
E_W = 4 * DIM * DIM           # 262144  wq, wk, wv, wp (each (kt p) c row-major)
OFF_MK = 0
OFF_RPB = OFF_MK + E_MK
OFF_W = OFF_RPB + E_RPB
OFF_VB = OFF_W + E_W
OFF_PB = OFF_VB + 2 * N
OFF_ONES = OFF_PB + N
TOT_C = OFF_ONES + 128        # cblob length

LAST_RESULTS = {}


# ---------------------------------------------------------------- host helpers
def _ln_np(x, g, b):
    m = x.mean(-1, keepdims=True)
    v = ((x - m) ** 2).mean(-1, keepdims=True)
    return (x - m) / np.sqrt(v + 1e-5) * g + b


def _pos_bias_np(H, W, pp_w, pp_b, ln1_g, ln1_b, l1_w, l1_b, ln2_g, ln2_b,
                 l2_w, l2_b, ln3_g, ln3_b, l3_w, l3_b):
    bh = np.arange(1 - H, H, dtype=np.float32)
    bw = np.arange(1 - W, W, dtype=np.float32)
    mg = np.stack(np.meshgrid(bh, bw, indexing="ij"))
    biases = mg.reshape(2, -1).T
    x = biases @ pp_w.T + pp_b
    x = _ln_np(x, ln1_g, ln1_b)
    x = np.maximum(x, 0) @ l1_w.T + l1_b
    x = _ln_np(x, ln2_g, ln2_b)
    x = np.maximum(x, 0) @ l2_w.T + l2_b
    x = _ln_np(x, ln3_g, ln3_b)
    pos = np.maximum(x, 0) @ l3_w.T + l3_b  # (L, NH)
    ch = np.arange(H)
    cw = np.arange(W)
    coords = np.stack(np.meshgrid(ch, cw, indexing="ij")).reshape(2, -1)
    rel = coords[:, :, None] - coords[:, None, :]
    rel = rel.transpose(1, 2, 0) + np.array([H - 1, W - 1])
    idx = rel[..., 0] * (2 * W - 1) + rel[..., 1]
    rpb = pos[idx.reshape(-1)].reshape(H * W, H * W, -1)
    return rpb.transpose(2, 0, 1).astype(np.float32)  # (NH, N, N)


# ---------------------------------------------------------------- device kernel
def _build_kernel():
    nc = bacc.Bacc(
        "TRN2",
        target_bir_lowering=False,
        debug=False,
        enable_asserts=False,
        num_devices=NCORES,
    )

    cblob = nc.dram_tensor("cblob", [TOT_C], BF16, kind="ExternalInput").ap()
    kblob = nc.dram_tensor("kblob", [E_KT], BF16, kind="ExternalInput").ap()
    qblob = nc.dram_tensor("qblob", [E_QT], BF16, kind="ExternalInput").ap()
    dout = nc.dram_tensor("out", [WPC, N, DIM], F16, kind="ExternalOutput").ap()

    with ExitStack() as ctx:
        tc = ctx.enter_context(tile.TileContext(nc))
        # ---------------- persistent SBUF: weights + constants
        wpool = ctx.enter_context(tc.tile_pool(name="w", bufs=1))
        wq = wpool.tile([128, 2, DIM], BF16, tag="wq")
        wk = wpool.tile([128, 2, DIM], BF16, tag="wk")
        wv = wpool.tile([128, 2, DIM], BF16, tag="wv")
        wp = wpool.tile([128, 2, DIM], BF16, tag="wp")
        # layout: tile[p, kt, co] = W^T[kt*128+p, co]
        for i, t in enumerate((wq, wk, wv, wp)):
            nc.sync.dma_start(
                t[:],
                cblob[OFF_W + i * DIM * DIM: OFF_W + (i + 1) * DIM * DIM]
                .rearrange("(kt p c) -> p kt c", kt=2, p=128, c=DIM))
        vb_sb = wpool.tile([1, 2 * N], BF16, tag="vb")
        pb_sb = wpool.tile([1, N], BF16, tag="pb")
        ones_sb = wpool.tile([1, 128], BF16, tag="ones")
        nc.sync.dma_start(vb_sb[:], cblob[OFF_VB:OFF_VB + 2 * N]
                          .rearrange("(p x) -> p x", p=1))
        nc.sync.dma_start(pb_sb[:], cblob[OFF_PB:OFF_PB + N]
                          .rearrange("(p x) -> p x", p=1))
        nc.sync.dma_start(ones_sb[:], cblob[OFF_ONES:OFF_ONES + 128]
                          .rearrange("(p x) -> p x", p=1))

        # rpb tiles: [it][128 i, h*256 j]  (host stores rpb it-major)
        rpb_sb = [wpool.tile([128, NH * N], BF16, name=f"rpb{it}", tag=f"rpb{it}") for it in range(2)]
        for it in range(2):
            nc.sync.dma_start(
                rpb_sb[it][:],
                cblob[OFF_RPB + it * NH * 128 * N: OFF_RPB + (it + 1) * NH * 128 * N]
                .rearrange("(h p j) -> p h j", h=NH, p=128, j=N))
        # mask tiles per group: [g][it] [128 i, 256 j]
        mask_sb = [wpool.tile([128, 2, N], BF16, name=f"msk{g}", tag=f"msk{g}") for g in range(GPC)]
        for g in range(GPC):
            nc.sync.dma_start(
                mask_sb[g][:],
                cblob[OFF_MK + g * N * N: OFF_MK + (g + 1) * N * N]
                .rearrange("(it p j) -> p it j", it=2, p=128, j=N))

        # ---------------- E = exp(rpb + mask) per (g, it): [128, 8h*256]
        epool = ctx.enter_context(tc.tile_pool(name="E", bufs=1))
        comb_pool = ctx.enter_context(tc.tile_pool(name="comb", bufs=2))
        E_sb = [[epool.tile([128, NH * N], BF16, name=f"E{g}_{it}", tag=f"E{g}_{it}") for it in range(2)]
                for g in range(GPC)]
        for g in range(GPC):
            for it in range(2):
                comb = comb_pool.tile([128, NH * N], BF16, tag="comb")
                for h in range(NH):
                    nc.vector.tensor_add(
                        comb[:, h * N:(h + 1) * N],
                        rpb_sb[it][:, h * N:(h + 1) * N],
                        mask_sb[g][:, it, :],
                    )
                nc.scalar.activation(E_sb[g][it][:], comb[:],
                                     mybir.ActivationFunctionType.Exp)

        # ---------------- pools for the window loop
        qin_pool = ctx.enter_context(tc.tile_pool(name="qin", bufs=2))
        qtr_pool = ctx.enter_context(tc.tile_pool(name="qtr", bufs=2))
        proj_ps = ctx.enter_context(tc.tile_pool(name="pps", bufs=2, space="PSUM"))
        qk_ps = ctx.enter_context(tc.tile_pool(name="qkps", bufs=2, space="PSUM"))
        proj_sb = ctx.enter_context(tc.tile_pool(name="psb", bufs=2))
        s_ps = ctx.enter_context(tc.tile_pool(name="sps", bufs=1, space="PSUM"))
        p_sb = ctx.enter_context(tc.tile_pool(name="p", bufs=2))
        pn_sb = ctx.enter_context(tc.tile_pool(name="pn", bufs=2))
        pt_sb = ctx.enter_context(tc.tile_pool(name="pt", bufs=2))
        z_sb = ctx.enter_context(tc.tile_pool(name="z", bufs=2))
        x_sb = ctx.enter_context(tc.tile_pool(name="x", bufs=2))
        y_sb = ctx.enter_context(tc.tile_pool(name="y", bufs=2))

        AF = mybir.ActivationFunctionType
        ALU = mybir.AluOpType

        for w in range(WPC):
            g = w // 4  # 4 consecutive windows share a mask group

            # -- load token-major q, k [128 t, tt, 256 c]; transpose on-device
            # (DMA xbar) to channel-major [128 cin, kt, 256 t] so the host
            # marshal is a contiguous cast with no strided transpose pass
            qtok = qin_pool.tile([128, 2, N], BF16, tag="qtok")
            ktok = qin_pool.tile([128, 2, N], BF16, tag="ktok")
            nc.sync.dma_start(qtok[:], qblob[w * DIM * N:(w + 1) * DIM * N]
                              .rearrange("(tt p c) -> p tt c", tt=2, p=128, c=DIM))
            nc.sync.dma_start(ktok[:], kblob[w * DIM * N:(w + 1) * DIM * N]
                              .rearrange("(tt p c) -> p tt c", tt=2, p=128, c=DIM))
            qT = qtr_pool.tile([128, 2, N], BF16, tag="qT")
            kT = qtr_pool.tile([128, 2, N], BF16, tag="kT")
            for src, dst in ((qtok, qT), (ktok, kT)):
                for tt in range(2):
                    for ct in range(2):
                        nc.sync.dma_start_transpose(
                            dst[:, ct, tt * 128:(tt + 1) * 128],
                            src[:, tt, ct * 128:(ct + 1) * 128])

            # -- q/k projections per-head (M=32, operands at partition base 0)
            # psum [32 d, 4h x 256 t]; evict -> sbuf [32, 8h*256]
            qh = proj_sb.tile([32, NH * N], BF16, tag="qh")
            kh = proj_sb.tile([32, NH * N], BF16, tag="kh")
            for dst, wmat in ((qh, wq), (kh, wk)):
                for grp in range(2):
                    pp = qk_ps.tile([32, 4 * N], F32, tag="qk")
                    for hh in range(4):
                        h = grp * 4 + hh
                        for kt in range(2):
                            nc.tensor.matmul(
                                pp[:, hh * N:(hh + 1) * N],
                                wmat[:, kt, 32 * h:32 * (h + 1)],
                                (qT if dst is qh else kT)[:, kt, :],
                                start=(kt == 0), stop=(kt == 1))
                    nc.vector.tensor_copy(dst[:, grp * 4 * N:(grp + 1) * 4 * N], pp[:])

            # -- v projection token-major (M=128): lhsT = kT block
            vh_ps = proj_ps.tile([128, 2, N], F32, tag="pp")
            for jt in range(2):
                for kt in range(2):
                    nc.tensor.matmul(vh_ps[:, jt, :], kT[:, kt, jt * 128:(jt + 1) * 128],
                                     wv[:, kt, :], start=(kt == 0), stop=False)
                nc.tensor.matmul(vh_ps[:, jt, :], ones_sb[0:1, :],
                                 vb_sb[0:1, jt * N:(jt + 1) * N], start=False, stop=True)
            vh = proj_sb.tile([128, 2, N], BF16, tag="vh")
            nc.vector.tensor_copy(vh[:], vh_ps[:])

            # -- S = qh_h^T kh_h (K=32 at base 0); exp; fused xE-multiply + rowsum
            ptil = p_sb.tile([128, 2, NH * N], BF16, tag="ptil")
            pu = pn_sb.tile([128, 2, NH * N], BF16, tag="pu")
            zt = z_sb.tile([128, NH, 2], F32, tag="z")
            rz = z_sb.tile([128, NH, 2], F32, tag="rz")
            for it in range(2):
                for g2 in range(2):
                    sp = s_ps.tile([128, 4 * N], F32, tag="sp")
                    for hh in range(4):
                        h = g2 * 4 + hh
                        nc.tensor.matmul(
                            sp[:, hh * N:(hh + 1) * N],
                            qh[:, h * N + it * 128: h * N + (it + 1) * 128],
                            kh[:, h * N:(h + 1) * N],
                            start=True, stop=True)
                    nc.scalar.activation(
                        ptil[:, it, g2 * 4 * N:(g2 + 1) * 4 * N], sp[:], AF.Exp)
                for h in range(NH):
                    nc.vector.scalar_tensor_tensor(
                        out=pu[:, it, h * N:(h + 1) * N],
                        in0=ptil[:, it, h * N:(h + 1) * N],
                        scalar=1.0,
                        in1=E_sb[g][it][:, h * N:(h + 1) * N],
                        op0=ALU.mult, op1=ALU.mult,
                        accum_out=zt[:, h, it:it + 1])
            nc.vector.reciprocal(rz[:], zt[:])

            # -- normalize rows, then DMA-xbar transpose -> Pt [jt][128 j, h*256 i]
            pnt = pt_sb.tile([128, 2, NH * N], BF16, tag="pnt")
            for it in range(2):
                for h in range(NH):
                    nc.vector.tensor_scalar_mul(
                        pu[:, it, h * N:(h + 1) * N],
                        pu[:, it, h * N:(h + 1) * N],
                        rz[:, h, it:it + 1])
            for h in range(NH):
                for it in range(2):
                    for jt in range(2):
                        nc.sync.dma_start_transpose(
                            pnt[:, jt, h * N + it * 128: h * N + (it + 1) * 128],
                            pu[:, it, h * N + jt * 128: h * N + (jt + 1) * 128])

            # -- O^T col-packed (verified): psum [128 (4h x 32d), 2 g2 x 256 i]
            ot_ps = proj_ps.tile([128, 2, N], F32, tag="pp")
            for g2 in range(2):
                for hh in range(4):
                    h = g2 * 4 + hh
                    for jt in range(2):
                        nc.tensor.matmul(
                            ot_ps[32 * hh:32 * (hh + 1), g2, :],
                            vh[:, jt, 32 * h:32 * (h + 1)],
                            pnt[:, jt, h * N:(h + 1) * N],
                            start=(jt == 0), stop=(jt == 1),
                            tile_position=(0, 32 * hh))
            xt = x_sb.tile([128, 2, N], BF16, tag="xt")
            nc.vector.tensor_copy(xt[:], ot_ps[:])

            # -- out projection: Y [128 t(mt), 256 c] += X^T blocks @ wpT
            y_ps = proj_ps.tile([128, 2, N], F32, tag="pp")
            for mt in range(2):
                for kt in range(2):
                    nc.tensor.matmul(y_ps[:, mt, :],
                                     xt[:, kt, mt * 128:(mt + 1) * 128],
                                     wp[:, kt, :], start=(kt == 0), stop=False)
                nc.tensor.matmul(y_ps[:, mt, :], ones_sb[0:1, :], pb_sb[0:1, :],
                                 start=False, stop=True)
            yo = y_sb.tile([128, 2, N], F16, tag="yo")
            nc.vector.tensor_copy(yo[:], y_ps[:])
            nc.sync.dma_start(
                dout[w].rearrange("(mt p) c -> p mt c", p=128), yo[:])

    nc.compile()
    return nc


# ---------------------------------------------------------------- persistent dispatch
_STATE = {}


def _get_state():
    if _STATE:
        return _STATE
    nc = _build_kernel()
    bass2jax.install_neuronx_cc_hook()

    partition_name = nc.partition_id_tensor.name if nc.partition_id_tensor else None
    in_names, out_names, out_avals = [], [], []
    for alloc in nc.m.functions[0].allocations:
        if not isinstance(alloc, mybir.MemoryLocationSet):
            continue
        name = alloc.memorylocations[0].name
        if alloc.kind == "ExternalInput":
            if name != partition_name:
                in_names.append(name)
        elif alloc.kind == "ExternalOutput":
            out_names.append(name)
            out_avals.append(jax.core.ShapedArray(
                tuple(alloc.tensor_shape), mybir.dt.np(alloc.dtype)))
    n_params = len(in_names)
    n_outs = len(out_avals)
    in_names_all = in_names + out_names
    if partition_name is not None:
        in_names_all.append(partition_name)
    donate = tuple(range(n_params, n_params + n_outs))

    def _body(*args):
        operands = list(args)
        if partition_name is not None:
            operands.append(bass2jax.partition_id_tensor())
        outs = bass2jax._bass_exec_p.bind(
            *operands,
            out_avals=tuple(out_avals),
            in_names=tuple(in_names_all),
            out_names=tuple(out_names),
            lowering_input_output_aliases=(),
            sim_require_finite=True,
            sim_require_nnan=True,
            nc=nc,
        )
        return tuple(outs)

    devices = jax.devices()[:NCORES]
    mesh = Mesh(np.asarray(devices), ("core",))
    sharding = NamedSharding(mesh, PartitionSpec("core"))
    in_specs = (PartitionSpec("core"),) * (n_params + n_outs)
    out_specs = (PartitionSpec("core"),) * n_outs
    sharded = jax.jit(
        shard_map(_body, mesh=mesh, in_specs=in_specs, out_specs=out_specs,
                  check_rep=False),
        donate_argnums=donate, keep_unused=True,
    )
    # donated output buffers created on-device: nothing shipped over the wire.
    gshapes = [(NCORES * a.shape[0], *a.shape[1:]) for a in out_avals]
    gdtypes = [a.dtype for a in out_avals]
    zeros_fn = jax.jit(
        lambda: tuple(jnp.zeros(s, d) for s, d in zip(gshapes, gdtypes)),
        out_shardings=tuple(sharding for _ in out_avals))

    _STATE.update(nc=nc, sharded=sharded, zeros_fn=zeros_fn, sharding=sharding,
                  in_names=in_names, out_names=out_names, out_avals=out_avals)
    return _STATE


# ---------------------------------------------------------------- entry point
def kernel(**inputs):
    st = _get_state()
    t_start = time.time()

    q = np.asarray(inputs["q"], np.float32)
    k = np.asarray(inputs["k"], np.float32)
    mask = np.asarray(inputs["mask"], np.float32)
    H = int(inputs["H"]); W = int(inputs["W"])
    assert H == 16 and W == 16 and q.shape == (B_, N, DIM)

    scale = float(HD) ** -0.5
    q_w = np.asarray(inputs["q_w"], np.float32)
    kv_w = np.asarray(inputs["kv_w"], np.float32)
    kv_b = np.asarray(inputs["kv_b"], np.float32)
    proj_w = np.asarray(inputs["proj_w"], np.float32)
    proj_b = np.asarray(inputs["proj_b"], np.float32)

    # donated out buffers materialize on-device while the host marshals
    zeros = st["zeros_fn"]()

    # ---- cblob: mask | rpb | weights | biases (2.5 MiB/core) -> upload first
    rpb = _pos_bias_np(
        H, W, *[np.asarray(inputs[n], np.float32) for n in
                ("pp_w", "pp_b", "ln1_g", "ln1_b", "l1_w", "l1_b", "ln2_g", "ln2_b",
                 "l2_w", "l2_b", "ln3_g", "ln3_b", "l3_w", "l3_b")])
    m16 = mask.astype(NPBF16)
    rpb16 = np.ascontiguousarray(
        rpb.reshape(NH, 2, 128, N).transpose(1, 0, 2, 3)).astype(NPBF16)  # [it,h,p,j]
    w16 = np.empty((4, DIM, DIM), NPBF16)
    w16[0] = q_w.T * scale
    w16[1] = kv_w[:DIM].T
    w16[2] = kv_w[DIM:].T
    w16[3] = proj_w.T
    cblob = np.empty((NCORES, TOT_C), NPBF16)
    for c in range(NCORES):
        cblob[c, OFF_MK:OFF_MK + E_MK] = m16[GPC * c:GPC * (c + 1)].reshape(-1)
        cblob[c, OFF_RPB:OFF_RPB + E_RPB] = rpb16.reshape(-1)
        cblob[c, OFF_W:OFF_W + E_W] = w16.reshape(-1)
        cblob[c, OFF_VB:OFF_VB + 2 * N] = np.tile(kv_b[DIM:], 2)
        cblob[c, OFF_PB:OFF_PB + N] = proj_b
        cblob[c, OFF_ONES:OFF_ONES + 128] = 1.0
    cblob_d = jax.device_put(cblob.reshape(-1), st["sharding"])

    # ---- kT / qT blobs: token-major windows (4 MiB/core each; the device
    # transposes to channel-major). device_put is async, so marshalling each
    # next blob overlaps the previous wire transfer
    kblob = k[_PERM].astype(NPBF16)
    kblob_d = jax.device_put(kblob.reshape(-1), st["sharding"])
    qblob = q[_PERM].astype(NPBF16)
    qblob_d = jax.device_put(qblob.reshape(-1), st["sharding"])
    LAST_RESULTS["marshal_s"] = time.time() - t_start

    g = {"cblob": cblob_d, "kblob": kblob_d, "qblob": qblob_d}
    t0 = time.time()
    out_arrs = st["sharded"](*[g[n] for n in st["in_names"]], *zeros)

    # ---- pipelined fetch: async per-shard D2H, cast/scatter overlaps the wire
    out = np.empty((B_, N, DIM), np.float32)
    shards = sorted(out_arrs[0].addressable_shards,
                    key=lambda s: s.index[0].start or 0)
    for s in shards:
        s.data.copy_to_host_async()
    for c, s in enumerate(shards):
        a16 = np.asarray(s.data)  # (WPC, N, DIM) f16
        out[_PERM[c * WPC:(c + 1) * WPC]] = a16.astype(np.float32)
    LAST_RESULTS["dispatch_s"] = time.time() - t0
    LAST_RESULTS["total_s"] = time.time() - t_start
    LAST_RESULTS["res"] = None
    return out


# revision 13
# speedup vs baseline: 1.3855x; 1.3855x over previous
"""Trainium2 Bass kernel for windowed sparse attention with dynamic position bias.

Reference computation (B_=256 windows, N=256 tokens, DIM=256, NH=8 heads, hd=32):
  qh = (q @ q_w.T + q_b)  -> heads;  kh, vh from kv projection of k
  attn = softmax(qh*s @ kh^T + rpb[h] + mask[b%64]);  out = (attn @ vh) @ proj_w.T + proj_b

Sharding: 8 cores, core c handles mask groups g in [8c, 8c+8), windows b = g + 64k
(4 windows per group -> exp(bias) tiles reused 4x per core).

Device kernel (per core, 32 windows): bf16 matmuls / fp32 PSUM.
  - projections from channel-major qT/kT (host-marshalled layout)
  - S = qh^T k (S-layout [i, j]), ACT exp from PSUM
  - P*E bias-multiply fused with row-sum via DVE tensor_tensor_reduce
  - normalize by 1/rowsum, DMA-xbar transpose P -> Pt, O^T = vh^T-packed matmuls
  - out-proj with K=1 ones-matmul bias add; output stored f16 (halves the
    device->host wire traffic vs f32; |out| <= ~2 so f16 rounding is ~5e-4 rel).

The axon host<->device link (~70 MiB/s up, ~37 MiB/s down, serialized) is the
bottleneck -- device exec is ~0.12 s. So the dispatch layer is organized around
wire bytes and overlap:
  - the Bass module + jit(shard_map(bass_exec)) wrapper are built once per
    process and cached (no per-call rebuild/recompile);
  - inputs are packed into 3 bf16 blob tensors (consts+mask+rpb | kT | qT),
    each device_put as soon as marshalled -- device_put is async under axon,
    so marshalling core i+1 overlaps the wire transfer of blob i;
  - donated output buffers are created on-device (jnp.zeros), not shipped;
  - the f16 output is fetched per-shard with copy_to_host_async, overlapping
    the f16->f32 cast + window scatter of shard c with the wire transfer of
    shard c+1.
"""

import os
import time
from contextlib import ExitStack

import numpy as np
import ml_dtypes

import jax
import jax.numpy as jnp
from jax.sharding import Mesh, PartitionSpec, NamedSharding
from jax.experimental.shard_map import shard_map  # same shim bass2jax uses

import concourse.bass as bass
import concourse.tile as tile
import concourse.mybir as mybir
from concourse import bacc
from concourse import bass2jax

BF16 = mybir.dt.bfloat16
F32 = mybir.dt.float32
F16 = mybir.dt.float16
INT8 = mybir.dt.int8
NPBF16 = ml_dtypes.bfloat16

DIM = 256
NH = 8
HD = DIM // NH  # 32
B_ = 256
N = 256
NG = 64
NCORES = 8
GPC = NG // NCORES  # 8 groups per core
WPC = B_ // NCORES  # 32 windows per core
PD = DIM // 16  # 16

# window order: core c handles mask groups [8c, 8c+8); window b = g + 64k
_PERM = np.concatenate([
    ((np.arange(GPC) + GPC * c)[:, None] + NG * np.arange(4)[None, :]).reshape(-1)
    for c in range(NCORES)
])  # (256,) global row i of the concatenated per-core input = window _PERM[i]

# ---- packed input blobs (bf16 elements, per-core) ----
E_QT = WPC * DIM * N          # 2097152
E_KT = E_QT
E_MK = GPC * N * N            # 524288
E_RPB = NH * N * N            # 524288  stored [it, h, 128, j]
E_W = 4 * DIM * DIM           # 262144  wq, wk, wv, wp (each (kt p) c row-major)
OFF_MK = 0
OFF_RPB = OFF_MK + E_MK
OFF_W = OFF_RPB + E_RPB
OFF_VB = OFF_W + E_W
OFF_PB = OFF_VB + 2 * N
OFF_ONES = OFF_PB + N
TOT_C = OFF_ONES + 128        # cblob length

LAST_RESULTS = {}


# ---------------------------------------------------------------- host helpers
def _ln_np(x, g, b):
    m = x.mean(-1, keepdims=True)
    v = ((x - m) ** 2).mean(-1, keepdims=True)
    return (x - m) / np.sqrt(v + 1e-5) * g + b


def _pos_bias_np(H, W, pp_w, pp_b, ln1_g, ln1_b, l1_w, l1_b, ln2_g, ln2_b,
                 l2_w, l2_b, ln3_g, ln3_b, l3_w, l3_b):
    bh = np.arange(1 - H, H, dtype=np.float32)
    bw = np.arange(1 - W, W, dtype=np.float32)
    mg = np.stack(np.meshgrid(bh, bw, indexing="ij"))
    biases = mg.reshape(2, -1).T
    x = biases @ pp_w.T + pp_b
    x = _ln_np(x, ln1_g, ln1_b)
    x = np.maximum(x, 0) @ l1_w.T + l1_b
    x = _ln_np(x, ln2_g, ln2_b)
    x = np.maximum(x, 0) @ l2_w.T + l2_b
    x = _ln_np(x, ln3_g, ln3_b)
    pos = np.maximum(x, 0) @ l3_w.T + l3_b  # (L, NH)
    ch = np.arange(H)
    cw = np.arange(W)
    coords = np.stack(np.meshgrid(ch, cw, indexing="ij")).reshape(2, -1)
    rel = coords[:, :, None] - coords[:, None, :]
    rel = rel.transpose(1, 2, 0) + np.array([H - 1, W - 1])
    idx = rel[..., 0] * (2 * W - 1) + rel[..., 1]
    rpb = pos[idx.reshape(-1)].reshape(H * W, H * W, -1)
    return rpb.transpose(2, 0, 1).astype(np.float32)  # (NH, N, N)


# ---------------------------------------------------------------- device kernel
def _build_kernel():
    nc = bacc.Bacc(
        "TRN2",
        target_bir_lowering=False,
        debug=False,
        enable_asserts=False,
        num_devices=NCORES,
    )

    cblob = nc.dram_tensor("cblob", [TOT_C], BF16, kind="ExternalInput").ap()
    # q/k ship as int8 with per-(window,token) f32 scales: halves the dominant
    # wire cost; dequant on-device before the projections
    ki8 = nc.dram_tensor("ki8", [E_KT], INT8, kind="ExternalInput").ap()
    qi8 = nc.dram_tensor("qi8", [E_QT], INT8, kind="ExternalInput").ap()
    ksc = nc.dram_tensor("ksc", [WPC * N], F32, kind="ExternalInput").ap()
    qsc = nc.dram_tensor("qsc", [WPC * N], F32, kind="ExternalInput").ap()
    dout = nc.dram_tensor("out", [WPC, N, DIM], F16, kind="ExternalOutput").ap()

    with ExitStack() as ctx:
        tc = ctx.enter_context(tile.TileContext(nc))
        # ---------------- persistent SBUF: weights + constants
        wpool = ctx.enter_context(tc.tile_pool(name="w", bufs=1))
        wq = wpool.tile([128, 2, DIM], BF16, tag="wq")
        wk = wpool.tile([128, 2, DIM], BF16, tag="wk")
        wv = wpool.tile([128, 2, DIM], BF16, tag="wv")
        wp = wpool.tile([128, 2, DIM], BF16, tag="wp")
        # layout: tile[p, kt, co] = W^T[kt*128+p, co]
        for i, t in enumerate((wq, wk, wv, wp)):
            nc.sync.dma_start(
                t[:],
                cblob[OFF_W + i * DIM * DIM: OFF_W + (i + 1) * DIM * DIM]
                .rearrange("(kt p c) -> p kt c", kt=2, p=128, c=DIM))
        vb_sb = wpool.tile([1, 2 * N], BF16, tag="vb")
        pb_sb = wpool.tile([1, N], BF16, tag="pb")
        ones_sb = wpool.tile([1, 128], BF16, tag="ones")
        nc.sync.dma_start(vb_sb[:], cblob[OFF_VB:OFF_VB + 2 * N]
                          .rearrange("(p x) -> p x", p=1))
        nc.sync.dma_start(pb_sb[:], cblob[OFF_PB:OFF_PB + N]
                          .rearrange("(p x) -> p x", p=1))
        nc.sync.dma_start(ones_sb[:], cblob[OFF_ONES:OFF_ONES + 128]
                          .rearrange("(p x) -> p x", p=1))
        # dequant row scales: [128 t_lo, w, tt]
        qsc_sb = wpool.tile([128, WPC, 2], F32, tag="qsc")
        ksc_sb = wpool.tile([128, WPC, 2], F32, tag="ksc")
        nc.sync.dma_start(qsc_sb[:], qsc.rearrange("(w tt p) -> p w tt", w=WPC, tt=2, p=128))
        nc.sync.dma_start(ksc_sb[:], ksc.rearrange("(w tt p) -> p w tt", w=WPC, tt=2, p=128))

        # rpb tiles: [it][128 i, h*256 j]  (host stores rpb it-major)
        rpb_sb = [wpool.tile([128, NH * N], BF16, name=f"rpb{it}", tag=f"rpb{it}") for it in range(2)]
        for it in range(2):
            nc.sync.dma_start(
                rpb_sb[it][:],
                cblob[OFF_RPB + it * NH * 128 * N: OFF_RPB + (it + 1) * NH * 128 * N]
                .rearrange("(h p j) -> p h j", h=NH, p=128, j=N))
        # mask tiles per group: [g][it] [128 i, 256 j]
        mask_sb = [wpool.tile([128, 2, N], BF16, name=f"msk{g}", tag=f"msk{g}") for g in range(GPC)]
        for g in range(GPC):
            nc.sync.dma_start(
                mask_sb[g][:],
                cblob[OFF_MK + g * N * N: OFF_MK + (g + 1) * N * N]
                .rearrange("(it p j) -> p it j", it=2, p=128, j=N))

        # ---------------- E = exp(rpb + mask) per (g, it): [128, 8h*256]
        epool = ctx.enter_context(tc.tile_pool(name="E", bufs=1))
        comb_pool = ctx.enter_context(tc.tile_pool(name="comb", bufs=2))
        E_sb = [[epool.tile([128, NH * N], BF16, name=f"E{g}_{it}", tag=f"E{g}_{it}") for it in range(2)]
                for g in range(GPC)]
        for g in range(GPC):
            for it in range(2):
                comb = comb_pool.tile([128, NH * N], BF16, tag="comb")
                for h in range(NH):
                    nc.vector.tensor_add(
                        comb[:, h * N:(h + 1) * N],
                        rpb_sb[it][:, h * N:(h + 1) * N],
                        mask_sb[g][:, it, :],
                    )
                nc.scalar.activation(E_sb[g][it][:], comb[:],
                                     mybir.ActivationFunctionType.Exp)

        # ---------------- pools for the window loop
        qin_pool = ctx.enter_context(tc.tile_pool(name="qin", bufs=2))
        qde_pool = ctx.enter_context(tc.tile_pool(name="qde", bufs=2))
        qtr_pool = ctx.enter_context(tc.tile_pool(name="qtr", bufs=2))
        proj_ps = ctx.enter_context(tc.tile_pool(name="pps", bufs=2, space="PSUM"))
        qk_ps = ctx.enter_context(tc.tile_pool(name="qkps", bufs=2, space="PSUM"))
        proj_sb = ctx.enter_context(tc.tile_pool(name="psb", bufs=2))
        s_ps = ctx.enter_context(tc.tile_pool(name="sps", bufs=1, space="PSUM"))
        p_sb = ctx.enter_context(tc.tile_pool(name="p", bufs=2))
        pn_sb = ctx.enter_context(tc.tile_pool(name="pn", bufs=2))
        pt_sb = ctx.enter_context(tc.tile_pool(name="pt", bufs=2))
        z_sb = ctx.enter_context(tc.tile_pool(name="z", bufs=2))
        x_sb = ctx.enter_context(tc.tile_pool(name="x", bufs=2))
        y_sb = ctx.enter_context(tc.tile_pool(name="y", bufs=2))

        AF = mybir.ActivationFunctionType
        ALU = mybir.AluOpType

        for w in range(WPC):
            g = w // 4  # 4 consecutive windows share a mask group

            # -- load token-major int8 q, k [128 t, tt, 256 c]; dequant (cast +
            # per-token row scale) to bf16, then transpose on-device (DMA xbar)
            # to channel-major [128 cin, kt, 256 t]
            qi = qin_pool.tile([128, 2, N], INT8, tag="qi")
            ki = qin_pool.tile([128, 2, N], INT8, tag="ki")
            nc.sync.dma_start(qi[:], qi8[w * DIM * N:(w + 1) * DIM * N]
                              .rearrange("(tt p c) -> p tt c", tt=2, p=128, c=DIM))
            nc.sync.dma_start(ki[:], ki8[w * DIM * N:(w + 1) * DIM * N]
                              .rearrange("(tt p c) -> p tt c", tt=2, p=128, c=DIM))
            qtok = qde_pool.tile([128, 2, N], BF16, tag="qtok")
            ktok = qde_pool.tile([128, 2, N], BF16, tag="ktok")
            for src, dst, sc in ((qi, qtok, qsc_sb), (ki, ktok, ksc_sb)):
                for tt in range(2):
                    nc.vector.tensor_copy(dst[:, tt, :], src[:, tt, :])
                    nc.vector.tensor_scalar_mul(
                        dst[:, tt, :], dst[:, tt, :], sc[:, w, tt:tt + 1])
            qT = qtr_pool.tile([128, 2, N], BF16, tag="qT")
            kT = qtr_pool.tile([128, 2, N], BF16, tag="kT")
            for src, dst in ((qtok, qT), (ktok, kT)):
                for tt in range(2):
                    for ct in range(2):
                        nc.sync.dma_start_transpose(
                            dst[:, ct, tt * 128:(tt + 1) * 128],
                            src[:, tt, ct * 128:(ct + 1) * 128])

            # -- q/k projections per-head (M=32, operands at partition base 0)
            # psum [32 d, 4h x 256 t]; evict -> sbuf [32, 8h*256]
            qh = proj_sb.tile([32, NH * N], BF16, tag="qh")
            kh = proj_sb.tile([32, NH * N], BF16, tag="kh")
            for dst, wmat in ((qh, wq), (kh, wk)):
                for grp in range(2):
                    pp = qk_ps.tile([32, 4 * N], F32, tag="qk")
                    for hh in range(4):
                        h = grp * 4 + hh
                        for kt in range(2):
                            nc.tensor.matmul(
                                pp[:, hh * N:(hh + 1) * N],
                                wmat[:, kt, 32 * h:32 * (h + 1)],
                                (qT if dst is qh else kT)[:, kt, :],
                                start=(kt == 0), stop=(kt == 1))
                    nc.vector.tensor_copy(dst[:, grp * 4 * N:(grp + 1) * 4 * N], pp[:])

            # -- v projection token-major (M=128): lhsT = kT block
            vh_ps = proj_ps.tile([128, 2, N], F32, tag="pp")
            for jt in range(2):
                for kt in range(2):
                    nc.tensor.matmul(vh_ps[:, jt, :], kT[:, kt, jt * 128:(jt + 1) * 128],
                                     wv[:, kt, :], start=(kt == 0), stop=False)
                nc.tensor.matmul(vh_ps[:, jt, :], ones_sb[0:1, :],
                                 vb_sb[0:1, jt * N:(jt + 1) * N], start=False, stop=True)
            vh = proj_sb.tile([128, 2, N], BF16, tag="vh")
            nc.vector.tensor_copy(vh[:], vh_ps[:])

            # -- S = qh_h^T kh_h (K=32 at base 0); exp; fused xE-multiply + rowsum
            ptil = p_sb.tile([128, 2, NH * N], BF16, tag="ptil")
            pu = pn_sb.tile([128, 2, NH * N], BF16, tag="pu")
            zt = z_sb.tile([128, NH, 2], F32, tag="z")
            rz = z_sb.tile([128, NH, 2], F32, tag="rz")
            for it in range(2):
                for g2 in range(2):
                    sp = s_ps.tile([128, 4 * N], F32, tag="sp")
                    for hh in range(4):
                        h = g2 * 4 + hh
                        nc.tensor.matmul(
                            sp[:, hh * N:(hh + 1) * N],
                            qh[:, h * N + it * 128: h * N + (it + 1) * 128],
                            kh[:, h * N:(h + 1) * N],
                            start=True, stop=True)
                    nc.scalar.activation(
                        ptil[:, it, g2 * 4 * N:(g2 + 1) * 4 * N], sp[:], AF.Exp)
                for h in range(NH):
                    nc.vector.scalar_tensor_tensor(
                        out=pu[:, it, h * N:(h + 1) * N],
                        in0=ptil[:, it, h * N:(h + 1) * N],
                        scalar=1.0,
                        in1=E_sb[g][it][:, h * N:(h + 1) * N],
                        op0=ALU.mult, op1=ALU.mult,
                        accum_out=zt[:, h, it:it + 1])
            nc.vector.reciprocal(rz[:], zt[:])

            # -- normalize rows, then DMA-xbar transpose -> Pt [jt][128 j, h*256 i]
            pnt = pt_sb.tile([128, 2, NH * N], BF16, tag="pnt")
            for it in range(2):
                for h in range(NH):
                    nc.vector.tensor_scalar_mul(
                        pu[:, it, h * N:(h + 1) * N],
                        pu[:, it, h * N:(h + 1) * N],
                        rz[:, h, it:it + 1])
            for h in range(NH):
                for it in range(2):
                    for jt in range(2):
                        nc.sync.dma_start_transpose(
                            pnt[:, jt, h * N + it * 128: h * N + (it + 1) * 128],
                            pu[:, it, h * N + jt * 128: h * N + (jt + 1) * 128])

            # -- O^T col-packed (verified): psum [128 (4h x 32d), 2 g2 x 256 i]
            ot_ps = proj_ps.tile([128, 2, N], F32, tag="pp")
            for g2 in range(2):
                for hh in range(4):
                    h = g2 * 4 + hh
                    for jt in range(2):
                        nc.tensor.matmul(
                            ot_ps[32 * hh:32 * (hh + 1), g2, :],
                            vh[:, jt, 32 * h:32 * (h + 1)],
                            pnt[:, jt, h * N:(h + 1) * N],
                            start=(jt == 0), stop=(jt == 1),
                            tile_position=(0, 32 * hh))
            xt = x_sb.tile([128, 2, N], BF16, tag="xt")
            nc.vector.tensor_copy(xt[:], ot_ps[:])

            # -- out projection: Y [128 t(mt), 256 c] += X^T blocks @ wpT
            y_ps = proj_ps.tile([128, 2, N], F32, tag="pp")
            for mt in range(2):
                for kt in range(2):
                    nc.tensor.matmul(y_ps[:, mt, :],
                                     xt[:, kt, mt * 128:(mt + 1) * 128],
                                     wp[:, kt, :], start=(kt == 0), stop=False)
                nc.tensor.matmul(y_ps[:, mt, :], ones_sb[0:1, :], pb_sb[0:1, :],
                                 start=False, stop=True)
            yo = y_sb.tile([128, 2, N], F16, tag="yo")
            nc.vector.tensor_copy(yo[:], y_ps[:])
            nc.sync.dma_start(
                dout[w].rearrange("(mt p) c -> p mt c", p=128), yo[:])

    nc.compile()
    return nc


# ---------------------------------------------------------------- persistent dispatch
_STATE = {}


def _get_state():
    if _STATE:
        return _STATE
    nc = _build_kernel()
    bass2jax.install_neuronx_cc_hook()

    partition_name = nc.partition_id_tensor.name if nc.partition_id_tensor else None
    in_names, out_names, out_avals = [], [], []
    for alloc in nc.m.functions[0].allocations:
        if not isinstance(alloc, mybir.MemoryLocationSet):
            continue
        name = alloc.memorylocations[0].name
        if alloc.kind == "ExternalInput":
            if name != partition_name:
                in_names.append(name)
        elif alloc.kind == "ExternalOutput":
            out_names.append(name)
            out_avals.append(jax.core.ShapedArray(
                tuple(alloc.tensor_shape), mybir.dt.np(alloc.dtype)))
    n_params = len(in_names)
    n_outs = len(out_avals)
    in_names_all = in_names + out_names
    if partition_name is not None:
        in_names_all.append(partition_name)
    donate = tuple(range(n_params, n_params + n_outs))

    def _body(*args):
        operands = list(args)
        if partition_name is not None:
            operands.append(bass2jax.partition_id_tensor())
        outs = bass2jax._bass_exec_p.bind(
            *operands,
            out_avals=tuple(out_avals),
            in_names=tuple(in_names_all),
            out_names=tuple(out_names),
            lowering_input_output_aliases=(),
            sim_require_finite=True,
            sim_require_nnan=True,
            nc=nc,
        )
        return tuple(outs)

    devices = jax.devices()[:NCORES]
    mesh = Mesh(np.asarray(devices), ("core",))
    sharding = NamedSharding(mesh, PartitionSpec("core"))
    in_specs = (PartitionSpec("core"),) * (n_params + n_outs)
    out_specs = (PartitionSpec("core"),) * n_outs
    sharded = jax.jit(
        shard_map(_body, mesh=mesh, in_specs=in_specs, out_specs=out_specs,
                  check_rep=False),
        donate_argnums=donate, keep_unused=True,
    )
    # donated output buffers created on-device: nothing shipped over the wire.
    gshapes = [(NCORES * a.shape[0], *a.shape[1:]) for a in out_avals]
    gdtypes = [a.dtype for a in out_avals]
    zeros_fn = jax.jit(
        lambda: tuple(jnp.zeros(s, d) for s, d in zip(gshapes, gdtypes)),
        out_shardings=tuple(sharding for _ in out_avals))

    _STATE.update(nc=nc, sharded=sharded, zeros_fn=zeros_fn, sharding=sharding,
                  in_names=in_names, out_names=out_names, out_avals=out_avals)
    return _STATE


# ---------------------------------------------------------------- entry point
def kernel(**inputs):
    st = _get_state()
    t_start = time.time()

    q = np.asarray(inputs["q"], np.float32)
    k = np.asarray(inputs["k"], np.float32)
    mask = np.asarray(inputs["mask"], np.float32)
    H = int(inputs["H"]); W = int(inputs["W"])
    assert H == 16 and W == 16 and q.shape == (B_, N, DIM)

    scale = float(HD) ** -0.5
    q_w = np.asarray(inputs["q_w"], np.float32)
    kv_w = np.asarray(inputs["kv_w"], np.float32)
    kv_b = np.asarray(inputs["kv_b"], np.float32)
    proj_w = np.asarray(inputs["proj_w"], np.float32)
    proj_b = np.asarray(inputs["proj_b"], np.float32)

    # donated out buffers materialize on-device while the host marshals
    zeros = st["zeros_fn"]()

    # ---- cblob: mask | rpb | weights | biases (2.5 MiB/core) -> upload first
    rpb = _pos_bias_np(
        H, W, *[np.asarray(inputs[n], np.float32) for n in
                ("pp_w", "pp_b", "ln1_g", "ln1_b", "l1_w", "l1_b", "ln2_g", "ln2_b",
                 "l2_w", "l2_b", "ln3_g", "ln3_b", "l3_w", "l3_b")])
    m16 = mask.astype(NPBF16)
    rpb16 = np.ascontiguousarray(
        rpb.reshape(NH, 2, 128, N).transpose(1, 0, 2, 3)).astype(NPBF16)  # [it,h,p,j]
    w16 = np.empty((4, DIM, DIM), NPBF16)
    w16[0] = q_w.T * scale
    w16[1] = kv_w[:DIM].T
    w16[2] = kv_w[DIM:].T
    w16[3] = proj_w.T
    cblob = np.empty((NCORES, TOT_C), NPBF16)
    for c in range(NCORES):
        cblob[c, OFF_MK:OFF_MK + E_MK] = m16[GPC * c:GPC * (c + 1)].reshape(-1)
        cblob[c, OFF_RPB:OFF_RPB + E_RPB] = rpb16.reshape(-1)
        cblob[c, OFF_W:OFF_W + E_W] = w16.reshape(-1)
        cblob[c, OFF_VB:OFF_VB + 2 * N] = np.tile(kv_b[DIM:], 2)
        cblob[c, OFF_PB:OFF_PB + N] = proj_b
        cblob[c, OFF_ONES:OFF_ONES + 128] = 1.0
    cblob_d = jax.device_put(cblob.reshape(-1), st["sharding"])

    # ---- k / q: token-major windows, int8 with per-(window,token) row scales
    # (2 MiB/core each; the device dequants + transposes). device_put is async,
    # so marshalling each next blob overlaps the previous wire transfer
    def _quant_rows(x):
        m = np.maximum(np.abs(x).max(axis=-1), 1e-30)  # (B_, N)
        qq = np.rint(x * (127.0 / m)[..., None]).astype(np.int8)
        return qq, (m * (1.0 / 127.0)).astype(np.float32)

    ki8, ksc = _quant_rows(k[_PERM])
    ki8_d = jax.device_put(ki8.reshape(-1), st["sharding"])
    ksc_d = jax.device_put(ksc.reshape(-1), st["sharding"])
    qi8, qsc = _quant_rows(q[_PERM])
    qi8_d = jax.device_put(qi8.reshape(-1), st["sharding"])
    qsc_d = jax.device_put(qsc.reshape(-1), st["sharding"])
    LAST_RESULTS["marshal_s"] = time.time() - t_start

    g = {"cblob": cblob_d, "ki8": ki8_d, "qi8": qi8_d, "ksc": ksc_d, "qsc": qsc_d}
    t0 = time.time()
    out_arrs = st["sharded"](*[g[n] for n in st["in_names"]], *zeros)

    # ---- pipelined fetch: async per-shard D2H, cast/scatter overlaps the wire
    out = np.empty((B_, N, DIM), np.float32)
    shards = sorted(out_arrs[0].addressable_shards,
                    key=lambda s: s.index[0].start or 0)
    for s in shards:
        s.data.copy_to_host_async()
    for c, s in enumerate(shards):
        a16 = np.asarray(s.data)  # (WPC, N, DIM) f16
        out[_PERM[c * WPC:(c + 1) * WPC]] = a16.astype(np.float32)
    LAST_RESULTS["dispatch_s"] = time.time() - t0
    LAST_RESULTS["total_s"] = time.time() - t_start
    LAST_RESULTS["res"] = None
    return out
